# revision 1
# baseline (speedup 1.0000x reference)
"""Multi-head attention (RoPE + u-bias + bool mask) Trainium2 Bass kernel.

Contract: kernel(**inputs) takes FULL unsharded inputs (see shapes below),
shards batch across 8 NeuronCores (data parallel), runs one Bass/Tile
program per core, and gathers the full output.

Hardcoded problem shapes:
  query/key/value: (8, 1024, 1024) f32, mask: (8, 1024, 1024) bool,
  Wq/Wk/Wv/Wo: (1024, 1024) f32, bq/bk/bv/bo: (1024,) f32,
  u_bias: (16, 64) f32.  Output: (8, 1024, 1024) f32.
"""

import sys

if "/opt/trn_rl_repo" not in sys.path:
    sys.path.insert(0, "/opt/trn_rl_repo")

from contextlib import ExitStack

import ml_dtypes
import numpy as np

import concourse.bass as bass
from concourse import bacc
import concourse.tile as tile
from concourse import mybir
from concourse.bass_utils import run_bass_kernel_spmd

B, S, D, H, Dh = 8, 1024, 1024, 16, 64
P = 128
NT = D // P  # 8 partition-tiles along d
ST = S // P  # 8 tiles along s/t
HF = S // 2  # 512, half of s / matmul moving chunk
FP = mybir.dt.float32
BF = mybir.dt.bfloat16
FPR = mybir.dt.float32r
ROPE_BASE = 10000.0
AF = mybir.ActivationFunctionType
ALU = mybir.AluOpType

N_CORES = 8
import os
STAGES = int(os.environ.get("K_STAGES", "5"))

# fraction of mask-multiply tiles routed to GPSIMD instead of DVE
GPSIMD_MASK = True


def build_nc():
    nc = bacc.Bacc("TRN2", target_bir_lowering=False, debug=False)

    # DRAM I/O
    xqT = nc.dram_tensor("xqT", [D, S], FPR, kind="ExternalInput").ap()
    xkT = nc.dram_tensor("xkT", [D, S], FPR, kind="ExternalInput").ap()
    xvT = nc.dram_tensor("xvT", [D, S], FPR, kind="ExternalInput").ap()
    wqT = nc.dram_tensor("wqT", [D, D], FPR, kind="ExternalInput").ap()
    wkT = nc.dram_tensor("wkT", [D, D], FPR, kind="ExternalInput").ap()
    wvT = nc.dram_tensor("wvT", [D, D], FPR, kind="ExternalInput").ap()
    woT = nc.dram_tensor("woT", [D, D], FPR, kind="ExternalInput").ap()
    maskT = nc.dram_tensor("maskT", [S, S], BF, kind="ExternalInput").ap()
    costab = nc.dram_tensor("costab", [P, S], FP, kind="ExternalInput").ap()
    sintab = nc.dram_tensor("sintab", [P, S], FP, kind="ExternalInput").ap()
    # smalls[:, 0:8]=ucols, 8:16=bqcols, 16:24=bkcols
    smalls = nc.dram_tensor("smalls", [P, 24], FP, kind="ExternalInput").ap()
    pswap = nc.dram_tensor("pswap", [P, P], FPR, kind="ExternalInput").ap()
    # rows[0, 0:1024]=bv, 1024:2048=bo, 2048:2176=ones
    rows = nc.dram_tensor("rows", [1, 2 * D + P], FPR, kind="ExternalInput").ap()
    vones = nc.dram_tensor("vones", [P, ST * H], FPR, kind="ExternalInput").ap()
    out = nc.dram_tensor("out", [S, D], FP, kind="ExternalOutput").ap()
    rec_d = nc.dram_tensor("rec_scratch", [H, S], FP).ap()

    with tile.TileContext(nc) as tc, ExitStack() as ctx:
        persist = ctx.enter_context(tc.tile_pool(name="persist", bufs=1))

        # ---- persistent constants / state ----
        cos_sb = persist.tile([P, S], FP)
        nc.sync.dma_start(cos_sb[:], costab[:])
        sin_sb = persist.tile([P, S], FP)
        nc.sync.dma_start(sin_sb[:], sintab[:])
        smalls_sb = persist.tile([P, 24], FP)
        nc.sync.dma_start(smalls_sb[:], smalls[:])
        ucols_sb = smalls_sb[:, 0:8]
        bqcols_sb = smalls_sb[:, 8:16]
        bkcols_sb = smalls_sb[:, 16:24]
        pswap_sb = persist.tile([P, P], FPR)
        nc.sync.dma_start(pswap_sb[:], pswap[:])
        rows_sb = persist.tile([1, 2 * D + P], FPR)
        nc.sync.dma_start(rows_sb[:], rows[:])
        bvrow_sb = rows_sb[:, 0:D]
        borow_sb = rows_sb[:, D : 2 * D]
        ones_row = rows_sb[:, 2 * D : 2 * D + P]

        qb_sb = persist.tile([P, NT * S], FPR)  # rope(Q)^T + u   tile j at j*S
        kb_sb = persist.tile([P, NT * S], FPR)  # rope(K)^T
        # V augmented with ones column: [p, st, h, 0:64]=V, [.,.,.,64]=1
        vaug = persist.tile([P, ST * H * (Dh + 1)], FPR)
        vaug_v = vaug[:].rearrange("p (st h c) -> p st h c", st=ST, h=H)
        nc.sync.dma_start(vaug_v[:, :, :, Dh : Dh + 1], vones[:].rearrange("p (st h) -> p st h", st=ST)[:, :, :, None])
        ctxu = persist.tile([P, NT * S], FPR)  # ctx^T (normalized in place)
        den = persist.tile([H, S], FP)
        rec = persist.tile([H, S], FP)

        # ======== stage 1+2: Q^T / K^T projections + rope (pool A) ========
        with (
            tc.tile_pool(name="poolA", bufs=2) as pA,
            tc.tile_pool(name="psA", bufs=2, space="PSUM") as psA,
        ):
            for xT, wT, bcols, is_q in (
                (xqT, wqT, bqcols_sb, True),
                (xkT, wkT, bkcols_sb, False),
            ):
                dst = qb_sb if is_q else kb_sb
                for c in range(2):  # s-half
                    x_tiles = []
                    for k in range(NT):
                        xt = pA.tile([P, HF], FPR, tag="xin", bufs=9)
                        nc.gpsimd.dma_start(
                            xt[:], xT[k * P : (k + 1) * P, c * HF : (c + 1) * HF]
                        )
                        x_tiles.append(xt)
                    for j in range(NT):
                        raw = psA.tile([P, HF], FP, tag="proj", bufs=3)
                        for k in range(NT):
                            wt = pA.tile([P, P], FPR, tag="wblk", bufs=10)
                            nc.sync.dma_start(
                                wt[:], wT[k * P : (k + 1) * P, j * P : (j + 1) * P]
                            )
                            nc.tensor.matmul(
                                raw[:],
                                wt[:],
                                x_tiles[k][:],
                                start=(k == 0),
                                stop=(k == NT - 1),
                            )
                        # evacuate with per-partition bias (pre-rope)
                        q_raw = pA.tile([P, HF], FPR, tag="qraw", bufs=2)
                        nc.scalar.activation(
                            q_raw[:], raw[:], AF.Identity, bias=bcols[:, j : j + 1]
                        )
                        # partner-swap via permutation matmul
                        qp = psA.tile([P, HF], FP, tag="perm", bufs=2)
                        nc.tensor.matmul(
                            qp[:], pswap_sb, q_raw[:], start=True, stop=True
                        )
                        chalf = slice(c * HF, (c + 1) * HF)
                        t1 = pA.tile([P, HF], FP, tag="t1", bufs=2)
                        nc.vector.tensor_tensor(
                            t1[:], q_raw[:], cos_sb[:, chalf], op=ALU.mult
                        )
                        t2 = pA.tile([P, HF], FP, tag="t2", bufs=2)
                        nc.vector.tensor_tensor(
                            t2[:], qp[:], sin_sb[:, chalf], op=ALU.mult
                        )
                        dslice = dst[:, j * S + c * HF : j * S + (c + 1) * HF]
                        if is_q:
                            nc.vector.scalar_tensor_tensor(
                                dslice,
                                t1[:],
                                ucols_sb[:, j : j + 1],
                                t2[:],
                                op0=ALU.add,
                                op1=ALU.add,
                            )
                        else:
                            nc.vector.tensor_tensor(dslice, t1[:], t2[:], op=ALU.add)

        # ======== stages 3-5 (pool B) ========
        with tc.tile_pool(name="poolB", bufs=2) as pB:
          if STAGES >= 3:
            # ---- stage 3: V projection (untransposed) into vaug ----
            with tc.tile_pool(name="psV", bufs=2, space="PSUM") as psV:
              for st in range(ST):
                for c in range(2):
                    vp = psV.tile([P, HF], FP, tag="vproj", bufs=2)
                    for k in range(NT):
                        xb = pB.tile([P, P], FPR, tag="xvblk", bufs=4)
                        nc.sync.dma_start(
                            xb[:], xvT[k * P : (k + 1) * P, st * P : (st + 1) * P]
                        )
                        wm = pB.tile([P, HF], FPR, tag="wmov", bufs=4)
                        nc.sync.dma_start(
                            wm[:], wvT[k * P : (k + 1) * P, c * HF : (c + 1) * HF]
                        )
                        nc.tensor.matmul(
                            vp[:], xb[:], wm[:], start=(k == 0), stop=False
                        )
                    nc.tensor.matmul(
                        vp[:],
                        ones_row,
                        bvrow_sb[:, c * HF : (c + 1) * HF],
                        start=False,
                        stop=True,
                    )
                    nc.scalar.copy(
                        vaug_v[:, st, c * 8 : (c + 1) * 8, 0:Dh],
                        vp[:].rearrange("p (h c) -> p h c", h=8),
                    )

            # ---- stage 4: attention, head pairs interleaved over tt ----
          if STAGES >= 4:
            with tc.tile_pool(name="psAttn", bufs=2, space="PSUM") as psAt:
                for j in range(NT):
                    cps = [
                        psAt.tile([Dh + 1, S], FP, tag="ctx", bufs=2, name=f"cps{j}_{hi}")
                        for hi in range(2)
                    ]
                    for tt in range(ST):
                        mt = pB.tile([P, S], BF, tag="maskt", bufs=3)
                        nc.sync.dma_start(mt[:], maskT[tt * P : (tt + 1) * P, :])
                        for hi in range(2):
                            h = 2 * j + hi
                            half = hi * Dh
                            sps = psAt.tile([P, S], FP, tag="scores", bufs=2)
                            for c in range(2):
                                nc.tensor.matmul(
                                    sps[:, c * HF : (c + 1) * HF],
                                    
                                        kb_sb[
                                            half : half + Dh,
                                            j * S + tt * P : j * S + (tt + 1) * P,
                                        ]
                                    ,
                                    
                                        qb_sb[
                                            half : half + Dh,
                                            j * S + c * HF : j * S + (c + 1) * HF,
                                        ]
                                    ,
                                    start=True,
                                    stop=True,
                                )
                            et = pB.tile([P, S], FP, tag="expt", bufs=2)
                            nc.scalar.activation(et[:], sps[:], AF.Exp, scale=0.125)
                            em = pB.tile([P, S], FPR, tag="expm", bufs=2)
                            eng = (
                                nc.gpsimd
                                if (GPSIMD_MASK and (tt % 8 >= 5))
                                else nc.vector
                            )
                            eng.tensor_tensor(em[:], et[:], mt[:], op=ALU.mult)
                            for c in range(2):
                                nc.tensor.matmul(
                                    cps[hi][:, c * HF : (c + 1) * HF],
                                    vaug_v[:, tt, h, :],
                                    em[:, c * HF : (c + 1) * HF],
                                    start=(tt == 0),
                                    stop=(tt == ST - 1),
                                )
                    for hi in range(2):
                        h = 2 * j + hi
                        half = hi * Dh
                        nc.scalar.copy(
                            ctxu[half : half + Dh, j * S : (j + 1) * S],
                            cps[hi][0:Dh, :],
                        )
                        dstage = pB.tile([1, S], FP, tag="dstage", bufs=1)
                        nc.scalar.copy(dstage[:], cps[hi][Dh : Dh + 1, :])
                        nc.sync.dma_start(den[h : h + 1, :], dstage[:])

                # ---- stage 4b: normalize (in place) ----
                nc.vector.reciprocal(rec[:], den[:])
                nc.sync.dma_start(rec_d[:], rec[:])
                for j in range(NT):
                    rb = pB.tile([P, S], FP, tag="t1", bufs=2)
                    nc.sync.dma_start(
                        rb[0:Dh, :],
                        rec_d[2 * j : 2 * j + 1, :].to_broadcast([Dh, S]),
                    )
                    nc.sync.dma_start(
                        rb[Dh:P, :],
                        rec_d[2 * j + 1 : 2 * j + 2, :].to_broadcast([Dh, S]),
                    )
                    nc.vector.tensor_tensor(
                        ctxu[:, j * S : (j + 1) * S],
                        ctxu[:, j * S : (j + 1) * S],
                        rb[:],
                        op=ALU.mult,
                    )

            # ---- stage 5: output projection ----
          if STAGES >= 5:
            with tc.tile_pool(name="psO", bufs=2, space="PSUM") as psO:
                for st in range(ST):
                    op = psO.tile([P, S], FP, tag="oproj", bufs=2)
                    for c in range(2):
                        for k in range(NT):
                            wm = pB.tile([P, HF], FPR, tag="wmov", bufs=4)
                            nc.sync.dma_start(
                                wm[:],
                                woT[k * P : (k + 1) * P, c * HF : (c + 1) * HF],
                            )
                            nc.tensor.matmul(
                                op[:, c * HF : (c + 1) * HF],
                                ctxu[:, k * S + st * P : k * S + (st + 1) * P],
                                wm[:],
                                start=(k == 0),
                                stop=False,
                            )
                        nc.tensor.matmul(
                            op[:, c * HF : (c + 1) * HF],
                            ones_row,
                            borow_sb[:, c * HF : (c + 1) * HF],
                            start=False,
                            stop=True,
                        )
                    ot = pB.tile([P, S], FP, tag="ot", bufs=2)
                    nc.scalar.copy(ot[:], op[:])
                    nc.sync.dma_start(out[st * P : (st + 1) * P, :], ot[:])

    nc.compile()
    return nc


def _host_consts():
    inv_freq = 1.0 / (ROPE_BASE ** (np.arange(0, Dh, 2, dtype=np.float64) / Dh))
    freqs = np.arange(S, dtype=np.float64)[:, None] * inv_freq[None, :]  # [S, 32]
    cos_rep = np.repeat(np.cos(freqs), 2, axis=-1)  # [S, 64]
    sin_rep = np.repeat(np.sin(freqs), 2, axis=-1)
    costab = np.empty((P, S), np.float32)
    sintab = np.empty((P, S), np.float32)
    for p in range(P):
        dl = p % Dh
        costab[p, :] = cos_rep[:, dl]
        sgn = -1.0 if (p % 2 == 0) else 1.0
        sintab[p, :] = sgn * sin_rep[:, dl]
    pswap = np.zeros((P, P), np.float32)
    for k in range(P):
        pswap[k, k ^ 1] = 1.0
    return costab, sintab, pswap


def host_in_maps(query, key, value, mask, Wq, bq, Wk, bk, Wv, bv, u_bias, Wo, bo):
    costab, sintab, pswap = _host_consts()
    u = np.asarray(u_bias, np.float32)
    smalls = np.zeros((P, 24), np.float32)
    for j in range(NT):
        smalls[:, j] = np.concatenate([u[2 * j], u[2 * j + 1]])
    smalls[:, 8:16] = np.asarray(bq, np.float32).reshape(NT, P).T
    smalls[:, 16:24] = np.asarray(bk, np.float32).reshape(NT, P).T
    rows = np.concatenate(
        [np.asarray(bv, np.float32), np.asarray(bo, np.float32), np.ones(P, np.float32)]
    ).reshape(1, 2 * D + P)
    shared = dict(
        wqT=np.ascontiguousarray(np.asarray(Wq, np.float32).T),
        wkT=np.ascontiguousarray(np.asarray(Wk, np.float32).T),
        wvT=np.ascontiguousarray(np.asarray(Wv, np.float32).T),
        woT=np.ascontiguousarray(np.asarray(Wo, np.float32).T),
        costab=costab,
        sintab=sintab,
        smalls=smalls,
        pswap=pswap,
        rows=rows,
        vones=np.ones((P, ST * H), np.float32),
    )
    in_maps = []
    for b in range(N_CORES):
        m = dict(shared)
        m["xqT"] = np.ascontiguousarray(np.asarray(query[b], np.float32).T)
        m["xkT"] = np.ascontiguousarray(np.asarray(key[b], np.float32).T)
        m["xvT"] = np.ascontiguousarray(np.asarray(value[b], np.float32).T)
        m["maskT"] = np.ascontiguousarray(
            (~np.asarray(mask[b], bool)).T.astype(ml_dtypes.bfloat16)
        )
        in_maps.append(m)
    return in_maps


_CACHED = {}


def kernel(query, key, value, mask, Wq, bq, Wk, bk, Wv, bv, u_bias, Wo, bo):
    if "nc" not in _CACHED:
        _CACHED["nc"] = build_nc()
    nc = _CACHED["nc"]
    in_maps = host_in_maps(
        query, key, value, mask, Wq, bq, Wk, bk, Wv, bv, u_bias, Wo, bo
    )
    res = run_bass_kernel_spmd(nc, in_maps, list(range(N_CORES)))
    return np.stack([res.results[b]["out"] for b in range(N_CORES)], axis=0)

